# revision 1
# baseline (speedup 1.0000x reference)
"""CosHead kernel for Trainium2 (8 NeuronCores, data-parallel over batch).

Computes out[b,c,h,w] = 10 * scale[c] * cos_sim(x[b,:,h,w], weights[c,:])
 = (x[b,:,hw] . wn_scaled[c,:]) / ||x[b,:,hw]||
where wn_scaled[c,:] = weights[c,:] / ||weights[c,:]|| * scale[c] * 10.

Per-core plan (core b gets batch b; weights/scale replicated):
  - weight prep on device: normalize+scale [80,256], PE-transpose to [256,80]
  - stream x [256,16384] in 8 hw-tiles of 2048 cols:
      * one fused DMA load (both 128-partition d-chunks) per tile
      * squares for the norm path: chunk0 on ScalarE (Square, f32->bf16),
        chunk1 on GpSimd (tensor_mul) to balance engines
      * matmuls batched by stationary operand (fewer LDWEIGHTS switches):
        8 float32r gemm MMs (wnT stationary) -> 2x psum [80,1024], then
        8 bf16 norm MMs (ones [128,80] stationary -> column-sum broadcast
        to all 80 partitions, no separate broadcast step needed)
      * per 1024-half: ACT Sqrt(psum_n)->sbuf, DVE reciprocal_approx_fast,
        DVE tensor_mul(psum_g, inv) -> out tile; 1 gpsimd DMA store/tile
Measured floor: ~7us NEFF preamble + ~61us DMA (21.5MB at ~350GB/s,
read+write share the 16 SDMA engines) + tail + ~9us Tile exit barrier.
"""

import os
import sys

import numpy as np

for _p in ("/opt/trn_rl_repo",):
    if os.path.isdir(_p) and _p not in sys.path:
        sys.path.append(_p)

B, D, C = 8, 256, 80
HW = 128 * 128
TILE = 2048
SUB = 512
NT = HW // TILE
NS = TILE // SUB
P = 128  # SBUF partitions / d-chunk size
N_CORES = 8

_NC_CACHE = {}


def build_bass_kernel(hw: int = HW, tile_cols: int = TILE):
    """Build the single-core Bass program (SPMD: all cores run this)."""
    import concourse.bass as bass
    import concourse.tile as tile
    from concourse import bacc, mybir
    from concourse.masks import make_identity

    f32 = mybir.dt.float32
    f32r = mybir.dt.float32r
    bf16 = mybir.dt.bfloat16
    mult = mybir.AluOpType.mult

    nt = hw // tile_cols
    ns = tile_cols // SUB

    nc = bacc.Bacc("TRN2", target_bir_lowering=False, debug=False)
    x_d = nc.declare_dram_parameter("x", [D, hw], f32r, isOutput=False)
    w_d = nc.declare_dram_parameter("weights", [C, D], f32, isOutput=False)
    s_d = nc.declare_dram_parameter(
        "adaptive_scale_factor", [C], f32, isOutput=False
    )
    out_d = nc.declare_dram_parameter("out", [C, hw], f32, isOutput=True)

    with tile.TileContext(nc) as tc:
        with (
            tc.tile_pool(name="setup", bufs=1) as setup,
            tc.tile_pool(name="xp", bufs=3) as xp,
            tc.tile_pool(name="x2p", bufs=3) as x2p,
            tc.tile_pool(name="outp", bufs=6) as outp,
            tc.tile_pool(name="subp", bufs=4) as subp,
            tc.tile_pool(name="pg", bufs=2, space=bass.MemorySpace.PSUM) as pgp,
            tc.tile_pool(name="pn", bufs=4, space=bass.MemorySpace.PSUM) as pnp,
        ):
            # ---- weight prep (tiny, once) ----
            w_sb = setup.tile([C, D], f32)
            nc.gpsimd.dma_start(out=w_sb, in_=w_d[:, :])
            sc_sb = setup.tile([C, 1], f32)
            nc.gpsimd.dma_start(out=sc_sb, in_=s_d[:, None])

            wsq = setup.tile([C, D], f32)
            nc.vector.tensor_mul(wsq, w_sb, w_sb)
            wss = setup.tile([C, 1], f32)
            nc.vector.reduce_sum(wss, wsq, axis=mybir.AxisListType.X)
            wsqrt = setup.tile([C, 1], f32)
            nc.scalar.sqrt(wsqrt, wss)
            winv = setup.tile([C, 1], f32)
            nc.vector.reciprocal(winv, wsqrt)  # exact; [80,1] is tiny
            rs = setup.tile([C, 1], f32)
            nc.vector.tensor_mul(rs, winv, sc_sb)
            # wn = w * (1/||w||) * scale * 10
            wn = setup.tile([C, D], f32)
            nc.vector.tensor_scalar(
                wn, w_sb, scalar1=rs, scalar2=10.0, op0=mult, op1=mult
            )

            ident = setup.tile([P, P], f32)
            make_identity(nc, ident)

            wnT = []
            for k in range(D // P):
                pt = pnp.tile([P, C], f32, tag="pn")
                nc.tensor.transpose(pt, wn[:, k * P : (k + 1) * P], ident[:C, :C])
                t_sb = setup.tile([P, C], f32r, tag=f"wnT{k}")
                nc.vector.tensor_copy(t_sb, pt)
                wnT.append(t_sb)

            ones_sb = setup.tile([P, C], bf16)
            nc.vector.memset(ones_sb, 1.0)

            # ---- main loop over hw tiles ----
            # [256,hw] viewed as [128 partitions, 2 d-chunks, hw] so one
            # dma_start fetches both chunks; stores go via gpsimd so the
            # sync queue never blocks next tile's load on this tile's math
            x_src = x_d[:, :].rearrange("(c p) w -> p c w", c=2)
            for t in range(nt):
                lo = t * tile_cols
                hi = lo + tile_cols
                x_sb = xp.tile([P, 2 * tile_cols], f32r)
                nc.sync.dma_start(
                    out=x_sb[:].rearrange("p (c w) -> p c w", c=2),
                    in_=x_src[:, :, lo:hi],
                )

                x2_sb = x2p.tile([P, 2 * tile_cols], bf16)
                nc.scalar.square(x2_sb[:, :tile_cols], x_sb[:, :tile_cols].bitcast(f32))
                nc.gpsimd.tensor_mul(
                    x2_sb[:, tile_cols:],
                    x_sb[:, tile_cols:].bitcast(f32),
                    x_sb[:, tile_cols:].bitcast(f32),
                )

                out_sb = outp.tile([C, tile_cols], f32)
                # batch matmuls by stationary operand: one LDW group for
                # wnT0, one for wnT1 (accumulate), one for ones (norm).
                pgs = [
                    pgp.tile([C, 2 * SUB], f32, tag="pg", name=f"pg{_i}")
                    for _i in range(2)
                ]
                pns = [
                    pnp.tile([C, SUB], f32, tag="pn", name=f"pn{_i}")
                    for _i in range(ns)
                ]
                for si in range(ns):
                    a, b = si * SUB, (si + 1) * SUB
                    nc.tensor.matmul(
                        pgs[si // 2][:, (si % 2) * SUB : (si % 2 + 1) * SUB],
                        wnT[0],
                        x_sb[:, a:b],
                        start=True,
                        stop=False,
                    )
                for si in range(ns):
                    a, b = si * SUB, (si + 1) * SUB
                    nc.tensor.matmul(
                        pgs[si // 2][:, (si % 2) * SUB : (si % 2 + 1) * SUB],
                        wnT[1],
                        x_sb[:, tile_cols + a : tile_cols + b],
                        start=False,
                        stop=True,
                    )
                for si in range(ns):
                    a, b = si * SUB, (si + 1) * SUB
                    nc.tensor.matmul(
                        pns[si], ones_sb, x2_sb[:, a:b], start=True, stop=False
                    )
                    nc.tensor.matmul(
                        pns[si],
                        ones_sb,
                        x2_sb[:, tile_cols + a : tile_cols + b],
                        start=False,
                        stop=True,
                    )
                for hf in range(2):
                    sq = subp.tile([C, 2 * SUB], f32, tag="sq")
                    for sj in range(2):
                        nc.scalar.sqrt(
                            sq[:, sj * SUB : (sj + 1) * SUB], pns[2 * hf + sj]
                        )
                    inv = subp.tile([C, 2 * SUB], f32, tag="inv")
                    nc.vector.reciprocal_approx_fast(inv, sq)
                    nc.vector.tensor_mul(
                        out_sb[:, 2 * hf * SUB : 2 * (hf + 1) * SUB], pgs[hf], inv
                    )

                nc.gpsimd.dma_start(out=out_d[:, lo:hi], in_=out_sb)

    nc.compile()
    return nc


def kernel(x, weights, adaptive_scale_factor):
    from concourse.bass_utils import run_bass_kernel_spmd

    x = np.ascontiguousarray(x, dtype=np.float32)
    weights = np.ascontiguousarray(weights, dtype=np.float32)
    scale = np.ascontiguousarray(adaptive_scale_factor, dtype=np.float32)

    if "nc" not in _NC_CACHE:
        _NC_CACHE["nc"] = build_bass_kernel()
    nc = _NC_CACHE["nc"]

    in_maps = [
        {
            "x": x[b].reshape(D, HW),
            "weights": weights,
            "adaptive_scale_factor": scale,
        }
        for b in range(N_CORES)
    ]
    res = run_bass_kernel_spmd(nc, in_maps, core_ids=list(range(N_CORES)))
    out = np.stack(
        [res.results[b]["out"].reshape(C, 128, 128) for b in range(N_CORES)]
    )
    return out.astype(np.float32)

